# revision 1
# baseline (speedup 1.0000x reference)
"""Contrastive-center loss on 8 Trainium2 NeuronCores.

Math: with D[b,c] = ||feat_b - w_c||^2,
  intra = sum_b D[b, label_b]
  total = sum_{b,c} D[b,c] = C*sum_b||f_b||^2 + B*sum_c||w_c||^2
                             - 2*(sum_b f_b)·(sum_c w_c)
  inter = total - intra
  loss  = (1/2/B) * intra / (inter + eps) / 0.1

The full (B,C) distance matrix is never materialized. Per core (256-row
batch shard), with mask[b,c] = (label_b == c):
  S     = mask.T @ feat        (C,D)  class-sums of feats   [PE]
  cnt   = mask.T @ ones        (C,1)  label histogram       [PE]
  f2    = rowsum(feat^2)              via ACT square+accum
  sum_b f_b = colsum(S)               (every b has exactly one label)
  sum_b w_label_b · feat_b = sum(S * w)
  sum_b ||w_label_b||^2    = cnt · c2
Five shard scalars are AllGathered, reduced with a ones-matmul, and the
loss formula is evaluated identically on every core.
"""

import numpy as np

import concourse.bacc as bacc
import concourse.tile as tile
from concourse import mybir
from concourse.bass_utils import run_bass_kernel_spmd

B, C, D = 2048, 100, 512
N_CORES = 8
BS = B // N_CORES  # 256 batch rows per core
P = 128
NT = BS // P  # 2 partition tiles per core
LAMBDA_C = 1.0
EPSILON = 1e-6
SCALE = LAMBDA_C / 2.0 / B / 0.1

f32 = mybir.dt.float32
i32 = mybir.dt.int32
AF = mybir.ActivationFunctionType
ALU = mybir.AluOpType


def _emit(nc, tc, feat, weight, label, loss, use_collective=True):
    with (
        tc.tile_pool(name="singles", bufs=1) as singles,
        tc.tile_pool(name="work", bufs=2) as work,
        tc.tile_pool(name="pp", bufs=1, space="PSUM") as pp,
        tc.tile_pool(name="dram", bufs=1, space="DRAM") as dram,
    ):
        # --- constants ---
        ones_col = singles.tile([P, 1], f32)
        nc.vector.memset(ones_col[:], 1.0)
        iota_i = singles.tile([P, C], i32)
        nc.gpsimd.iota(iota_i[:], pattern=[[1, C]], base=0, channel_multiplier=0)
        iota_f = singles.tile([P, C], f32)
        nc.vector.tensor_copy(iota_f[:], iota_i[:])
        # combined columns: 0=f2 rowsums, 1=cnt*c2, 2=rowsum(S*w), 3=c2
        combined = singles.tile([P, 4], f32)
        nc.vector.memset(combined[:], 0.0)
        partials = singles.tile([1, 16], f32)
        nc.vector.memset(partials[:], 0.0)

        # --- weight-side (once) ---
        w_sb = singles.tile([C, D], f32)
        nc.sync.dma_start(out=w_sb[:], in_=weight[:, :])
        w_sq = singles.tile([C, D], f32)
        nc.scalar.activation(
            w_sq[:], w_sb[:], AF.Square, accum_out=combined[:C, 3:4]
        )
        colw_psum = pp.tile([1, D], f32)
        nc.tensor.matmul(colw_psum[:], ones_col[:C], w_sb[:], start=True, stop=True)
        colw_sb = singles.tile([1, D], f32)
        nc.vector.tensor_copy(colw_sb[:], colw_psum[:])

        # --- batch tiles ---
        lab_i = singles.tile([P, NT], i32)
        nc.sync.dma_start(
            out=lab_i[:],
            in_=label.rearrange("(t p) o -> p (t o)", p=P),
        )
        lab_f = singles.tile([P, NT], f32)
        nc.vector.tensor_copy(lab_f[:], lab_i[:])
        S_psum = pp.tile([C, D], f32)
        cnt_psum = pp.tile([C, 1], f32)
        colf_psum = pp.tile([1, D], f32)
        f2_parts = []
        for t in range(NT):
            feat_t = work.tile([P, D], f32, name="feat_t")
            nc.sync.dma_start(out=feat_t[:], in_=feat[t * P : (t + 1) * P, :])
            mask = work.tile([P, C], f32, name="mask")
            nc.vector.tensor_scalar(
                mask[:], iota_f[:], lab_f[:, t : t + 1], None, op0=ALU.is_equal
            )
            sq = work.tile([P, D], f32, name="sq")
            f2p = work.tile([P, 1], f32, name="f2p")
            nc.scalar.activation(sq[:], feat_t[:], AF.Square, accum_out=f2p[:])
            f2_parts.append(f2p)
            nc.tensor.matmul(
                S_psum[:], mask[:], feat_t[:], start=(t == 0), stop=(t == NT - 1)
            )
            nc.tensor.matmul(
                cnt_psum[:], mask[:], ones_col[:], start=(t == 0), stop=(t == NT - 1)
            )
            nc.tensor.matmul(
                colf_psum[:], ones_col[:], feat_t[:], start=(t == 0), stop=(t == NT - 1)
            )

        # --- per-shard scalar reduction ---
        nc.vector.tensor_add(combined[:, 0:1], f2_parts[0][:], f2_parts[1][:])
        nc.vector.tensor_mul(w_sq[:], S_psum[:], w_sb[:])  # dead scratch, reused
        nc.vector.tensor_reduce(
            combined[:C, 2:3], w_sq[:], axis=mybir.AxisListType.X, op=ALU.add
        )
        nc.vector.tensor_mul(combined[:C, 1:2], cnt_psum[:], combined[:C, 3:4])
        dot_scr = singles.tile([1, D], f32)
        nc.vector.tensor_mul(dot_scr[:], colf_psum[:], colw_sb[:])
        dot_sb = singles.tile([1, 1], f32)
        nc.vector.tensor_reduce(
            dot_sb[:], dot_scr[:], axis=mybir.AxisListType.X, op=ALU.add
        )
        res_psum = pp.tile([1, 4], f32)
        nc.tensor.matmul(res_psum[:], ones_col[:], combined[:], start=True, stop=True)
        res_sb = singles.tile([1, 4], f32)
        nc.vector.tensor_copy(res_sb[:], res_psum[:])
        # res cols: 0=sum f2, 1=sum cnt*c2, 2=sum S*w, 3=sum c2 (all shard-local)
        # partials: [intra_shard, total_shard, ...]
        sc = singles.tile([1, 4], f32)
        nc.vector.tensor_add(sc[:, 0:1], res_sb[:, 0:1], res_sb[:, 1:2])
        nc.vector.tensor_scalar(
            partials[:, 0:1], res_sb[:, 2:3], -2.0, sc[:, 0:1],
            op0=ALU.mult, op1=ALU.add,
        )  # intra_shard
        nc.vector.tensor_scalar(
            sc[:, 1:2], res_sb[:, 0:1], float(C), None, op0=ALU.mult
        )
        nc.vector.tensor_scalar(
            sc[:, 2:3], res_sb[:, 3:4], float(BS), sc[:, 1:2],
            op0=ALU.mult, op1=ALU.add,
        )
        nc.vector.tensor_scalar(
            partials[:, 1:2], dot_sb[:], -2.0, sc[:, 2:3],
            op0=ALU.mult, op1=ALU.add,
        )  # total_shard

        # --- gather the two shard scalars across the 8 cores ---
        cc_in = dram.tile([1, 16], f32, name="cc_in")
        cc_out = dram.tile([N_CORES, 16], f32, addr_space="Shared", name="cc_out")
        nc.sync.dma_start(out=cc_in[:], in_=partials[:])
        if use_collective:
            nc.gpsimd.collective_compute(
                "AllGather",
                ALU.bypass,
                replica_groups=[list(range(N_CORES))],
                ins=[cc_in[:].opt()],
                outs=[cc_out[:].opt()],
            )
        else:
            nc.sync.dma_start(out=cc_out[0:1, :], in_=cc_in[:])
        ag_sb = singles.tile([N_CORES, 2], f32)
        nc.sync.dma_start(out=ag_sb[:], in_=cc_out[:, 0:2])
        G_psum = pp.tile([1, 2], f32)
        nc.tensor.matmul(G_psum[:], ones_col[:N_CORES], ag_sb[:], start=True, stop=True)
        g_sb = singles.tile([1, 2], f32)
        nc.vector.tensor_copy(g_sb[:], G_psum[:])

        # --- loss formula (identical on every core) ---
        INTRA, TOTAL = g_sb[:, 0:1], g_sb[:, 1:2]
        den = singles.tile([1, 2], f32)
        nc.vector.tensor_scalar(
            den[:, 0:1], TOTAL, INTRA, EPSILON, op0=ALU.subtract, op1=ALU.add
        )  # inter + eps
        nc.vector.reciprocal(den[:, 1:2], den[:, 0:1])
        loss_sb = singles.tile([1, 1], f32)
        nc.vector.tensor_scalar(
            loss_sb[:], den[:, 1:2], INTRA, SCALE, op0=ALU.mult, op1=ALU.mult
        )
        nc.sync.dma_start(out=loss[:, :], in_=loss_sb[:])


def build_bass(use_collective=True, reps=1):
    nc = bacc.Bacc(None, target_bir_lowering=False, num_devices=N_CORES)
    feat = nc.dram_tensor("feat", [BS, D], f32, kind="ExternalInput")
    weight = nc.dram_tensor("weight", [C, D], f32, kind="ExternalInput")
    label = nc.dram_tensor("label", [BS, 1], i32, kind="ExternalInput")
    loss = nc.dram_tensor("loss", [1, 1], f32, kind="ExternalOutput")
    with tile.TileContext(nc) as tc:
        for _ in range(reps):
            _emit(
                nc, tc, feat[:, :], weight[:, :], label[:, :], loss[:, :],
                use_collective=use_collective,
            )
    nc.compile()
    return nc


_NC = None


def _get_nc():
    global _NC
    if _NC is None:
        _NC = build_bass()
    return _NC


def make_in_maps(feat, weight, label):
    feat = np.ascontiguousarray(np.asarray(feat), dtype=np.float32)
    weight = np.ascontiguousarray(np.asarray(weight), dtype=np.float32)
    lab = np.ascontiguousarray(np.asarray(label).astype(np.int32).reshape(B, 1))
    return [
        {
            "feat": feat[c * BS : (c + 1) * BS],
            "weight": weight,
            "label": lab[c * BS : (c + 1) * BS],
        }
        for c in range(N_CORES)
    ]


def kernel(feat, weight, label):
    nc = _get_nc()
    in_maps = make_in_maps(feat, weight, label)
    res = run_bass_kernel_spmd(nc, in_maps, list(range(N_CORES)))
    return np.asarray(res.results[0]["loss"], dtype=np.float32).reshape(())

